# revision 18
# baseline (speedup 1.0000x reference)
"""Equivariant block-diagonal linear (128x0e+128x1o+64x2e+32x3o) on 8 trn2 cores.

Strategy (mode "bd", default):
  - Data-parallel: x [50000, 1056] row-sharded into 8x [6250, 1056].
  - Host de-interleaves each irrep: feature (off + u*d + i) -> row (off + i*mul + u)
    and transposes, so the device sees xt [1056, SHARD] where every irrep
    component i occupies a contiguous [mul, SHARD] row band.
  - Each 128-row chunk of xt then needs exactly ONE matmul with a small
    block-diagonal stationary blockdiag(w_r, ...) [fw, fw] - no accumulation
    chains, no kron(w, I_d) padding.
  - Per col-block of 1024 nodes: 9 input DMAs (4KB/partition lines), 9 matmuls
    per 512-col group, PSUM->SBUF copies on Vector, 9 output DMAs on Scalar.
    Input DMA issue on Sync/GpSimd so prefetch never queues behind outputs.
  - Host re-interleaves the gathered [1056, SHARD] outputs.
"""

import os
from contextlib import ExitStack

import numpy as np

import concourse.bass as bass
import concourse.tile as tile
from concourse import bacc, mybir
from concourse.bass_utils import run_bass_kernel_spmd

N_NODES = 50000
DIM = 1056
N_CORES = 8
SHARD = N_NODES // N_CORES  # 6250
P = 128

IRREPS = [(128, 0), (128, 1), (64, 2), (32, 3)]
# (offset, mul, d) per irrep in the feature axis
SEGS = [(0, 128, 1), (128, 128, 3), (512, 64, 5), (832, 32, 7)]

USE_FP32R = os.environ.get("KERNEL_FP32R", "1") == "1"
# device I/O + matmul dtype: bf16 halves DMA bytes (kernel is HBM-bound);
# rel err ~3e-3 vs the 2e-2 gate. "fp32r" keeps full-precision I/O.
DT = os.environ.get("KERNEL_DT", "bf16")

_cache = {}

# ---------------------------------------------------------------------------
# Mode "bd": de-interleaved rows; one matmul per 128-row chunk with a
# block-diagonal stationary.
# (row_off, rows, stationary_idx) - stationary s{si}[:rows, :rows]
CHUNKS_BD = [
    (0, 128, 0),
    (128, 128, 1),
    (256, 128, 1),
    (384, 128, 1),
    (512, 128, 2),
    (640, 128, 2),
    (768, 64, 2),
    (832, 128, 3),
    (960, 96, 3),
]
C_BLK = int(os.environ.get("KERNEL_CBLK", "2048" if DT == "bf16" else "1024"))


def _blocks():
    # small last block -> short post-input drain tail.
    sizes = None
    if sizes is None or sum(sizes) != SHARD:
        sizes = []
        left = SHARD
        while left > 0:
            sizes.append(min(C_BLK, left))
            left -= sizes[-1]
    out = []
    c0 = 0
    for cw in sizes:
        out.append((c0, cw))
        c0 += cw
    return out


def _build_bd():
    key = ("bd", DT, USE_FP32R, C_BLK)
    if key in _cache:
        return _cache[key]
    f32 = mybir.dt.float32
    f32r = mybir.dt.float32r
    bf16 = mybir.dt.bfloat16
    if DT == "bf16":
        iodt = bf16  # dram + sbuf dtype
        mmdt = bf16
    else:
        iodt = f32
        mmdt = f32r if USE_FP32R else f32
    nc = bacc.Bacc(
        "TRN2", target_bir_lowering=False, debug=False, num_devices=N_CORES
    )
    xt_d = nc.dram_tensor("xt", [DIM, SHARD], iodt, kind="ExternalInput")
    s_d = [
        nc.dram_tensor(f"s{r}", [P, P], iodt, kind="ExternalInput") for r in range(4)
    ]
    out_d = nc.dram_tensor("outt", [DIM, SHARD], iodt, kind="ExternalOutput")

    with ExitStack() as ctx:
        tc = ctx.enter_context(tile.TileContext(nc))
        wpool = ctx.enter_context(tc.tile_pool(name="w", bufs=1))
        xpool = ctx.enter_context(tc.tile_pool(name="xin", bufs=2))
        opool = ctx.enter_context(tc.tile_pool(name="oout", bufs=2))
        # 2-bank psum tiles: two matmuls fill halves, one wide copy drains
        pspool = ctx.enter_context(tc.tile_pool(name="ps", bufs=4, space="PSUM"))

        ssb = []
        for r in range(4):
            t = wpool.tile([P, P], mmdt, tag=f"s{r}")
            if mmdt == f32r:
                stg = wpool.tile([P, P], f32, tag=f"sstg{r}")
                nc.scalar.dma_start(stg[:], s_d[r][:])
                nc.vector.tensor_copy(t[:], stg[:])
            else:
                nc.scalar.dma_start(t[:], s_d[r][:])
            ssb.append(t)

        ci = 0  # copy round-robin Vector/Scalar
        for bi, (c0, cw) in enumerate(_blocks()):
            xins = []
            for k, (ro, fw, si) in enumerate(CHUNKS_BD):
                xin = xpool.tile([P, C_BLK], mmdt, tag=f"x{k}")
                src = xt_d[ro : ro + fw, c0 : c0 + cw]
                # block 0: flood all 3 DMA-issue engines to cut the ramp;
                # afterwards input stays on Sync (keeps prefetch decoupled
                # from output-side dependencies).
                if bi == 0:
                    eng = (nc.sync, nc.gpsimd, nc.scalar)[k % 3]
                else:
                    eng = nc.sync
                eng.dma_start(
                    xin[:fw, :cw], src.bitcast(mmdt) if mmdt == f32r else src
                )
                xins.append(xin)
            for k, (ro, fw, si) in enumerate(CHUNKS_BD):
                xout = opool.tile([P, C_BLK], iodt, tag=f"o{k}")
                for t0 in range(0, cw, 1024):
                    tw = min(1024, cw - t0)
                    ps = pspool.tile([P, 1024], f32, tag="ps")
                    for g0 in range(0, tw, 512):
                        gw = min(512, tw - g0)
                        nc.tensor.matmul(
                            ps[:fw, g0 : g0 + gw],
                            ssb[si][:fw, :fw],
                            xins[k][:fw, t0 + g0 : t0 + g0 + gw],
                            start=True,
                            stop=True,
                        )
                    if ci % 2 == 0:
                        nc.vector.tensor_copy(
                            xout[:fw, t0 : t0 + tw], ps[:fw, :tw]
                        )
                    else:
                        nc.scalar.copy(xout[:fw, t0 : t0 + tw], ps[:fw, :tw])
                    ci += 1
                oeng = nc.scalar if k % 2 == 0 else nc.gpsimd
                oeng.dma_start(
                    out_d[ro : ro + fw, c0 : c0 + cw], xout[:fw, :cw]
                )

    nc.compile()
    _cache[key] = nc
    return nc


def _deinterleave(xs):
    """[S, 1056] -> de-interleaved, transposed [1056, S] (keeps dtype)."""
    out = np.empty((DIM, xs.shape[0]), xs.dtype)
    for off, m, d in SEGS:
        out[off : off + m * d] = (
            xs[:, off : off + m * d].reshape(-1, m, d).transpose(2, 1, 0).reshape(m * d, -1)
        )
    return out


def _reinterleave(ot):
    """[1056, S] de-interleaved -> natural f32 [S, 1056]."""
    S = ot.shape[1]
    out = np.empty((S, DIM), np.float32)
    for off, m, d in SEGS:
        out[:, off : off + m * d] = (
            ot[off : off + m * d]
            .reshape(d, m, S)
            .transpose(2, 1, 0)
            .reshape(S, m * d)
            .astype(np.float32)
        )
    return out


def _stationaries(ws, dtype):
    reps = [1, 1, 2, 4]
    out = []
    for w, k in zip(ws, reps):
        w = np.asarray(w, np.float32)
        m = w.shape[0]
        bd = np.zeros((P, P), np.float32)
        for a in range(k):
            bd[a * m : (a + 1) * m, a * m : (a + 1) * m] = w
        out.append(np.ascontiguousarray(bd.astype(dtype)))
    return out


last_result = None  # BassKernelResults of the most recent run (for profiling)

MODE = os.environ.get("KERNEL_MODE", "bd")


def kernel(x, w0, w1, w2, w3):
    global last_result
    x = np.asarray(x, dtype=np.float32)
    trace = os.environ.get("KERNEL_TRACE", "0") == "1"
    assert MODE == "bd", MODE
    if DT == "bf16":
        from ml_dtypes import bfloat16

        hostdt = bfloat16
    else:
        hostdt = np.float32
    nc = _build_bd()
    ss = _stationaries([w0, w1, w2, w3], hostdt)
    if hostdt is not np.float32:
        x = x.astype(hostdt)
    in_maps = []
    for c in range(N_CORES):
        m = {"xt": _deinterleave(x[c * SHARD : (c + 1) * SHARD])}
        for r in range(4):
            m[f"s{r}"] = ss[r]
        in_maps.append(m)
    last_result = run_bass_kernel_spmd(
        nc, in_maps, core_ids=list(range(N_CORES)), trace=trace
    )
    return np.ascontiguousarray(
        np.concatenate(
            [_reinterleave(r["outt"]) for r in last_result.results], axis=0
        )
    )


# revision 19
# speedup vs baseline: 1.0943x; 1.0943x over previous
"""Equivariant block-diagonal linear (128x0e+128x1o+64x2e+32x3o) on 8 trn2 cores.

Strategy (mode "bd", default):
  - Data-parallel: x [50000, 1056] row-sharded into 8x [6250, 1056].
  - Host de-interleaves each irrep: feature (off + u*d + i) -> row (off + i*mul + u)
    and transposes, so the device sees xt [1056, SHARD] where every irrep
    component i occupies a contiguous [mul, SHARD] row band.
  - Each 128-row chunk of xt then needs exactly ONE matmul with a small
    block-diagonal stationary blockdiag(w_r, ...) [fw, fw] - no accumulation
    chains, no kron(w, I_d) padding.
  - Per col-block of 1024 nodes: 9 input DMAs (4KB/partition lines), 9 matmuls
    per 512-col group, PSUM->SBUF copies on Vector, 9 output DMAs on Scalar.
    Input DMA issue on Sync/GpSimd so prefetch never queues behind outputs.
  - Host re-interleaves the gathered [1056, SHARD] outputs.
"""

import os
from contextlib import ExitStack

import numpy as np

import concourse.bass as bass
import concourse.tile as tile
from concourse import bacc, mybir
from concourse.bass_utils import run_bass_kernel_spmd

N_NODES = 50000
DIM = 1056
N_CORES = 8
SHARD = N_NODES // N_CORES  # 6250
P = 128

IRREPS = [(128, 0), (128, 1), (64, 2), (32, 3)]
# (offset, mul, d) per irrep in the feature axis
SEGS = [(0, 128, 1), (128, 128, 3), (512, 64, 5), (832, 32, 7)]

USE_FP32R = os.environ.get("KERNEL_FP32R", "1") == "1"
# device I/O + matmul dtype: bf16 halves DMA bytes (kernel is HBM-bound);
# rel err ~3e-3 vs the 2e-2 gate. "fp32r" keeps full-precision I/O.
DT = os.environ.get("KERNEL_DT", "bf16")

_cache = {}

# ---------------------------------------------------------------------------
# Mode "bd": de-interleaved rows; one matmul per 128-row chunk with a
# block-diagonal stationary.
# (row_off, rows, stationary_idx) - stationary s{si}[:rows, :rows]
CHUNKS_BD = [
    (0, 128, 0),
    (128, 128, 1),
    (256, 128, 1),
    (384, 128, 1),
    (512, 128, 2),
    (640, 128, 2),
    (768, 64, 2),
    (832, 128, 3),
    (960, 96, 3),
]
C_BLK = int(os.environ.get("KERNEL_CBLK", "2048" if DT == "bf16" else "1024"))


def _blocks():
    # small last block -> short post-input drain tail.
    sizes = None
    if sizes is None or sum(sizes) != SHARD:
        sizes = []
        left = SHARD
        while left > 0:
            sizes.append(min(C_BLK, left))
            left -= sizes[-1]
    out = []
    c0 = 0
    for cw in sizes:
        out.append((c0, cw))
        c0 += cw
    return out


def _build_bd():
    key = ("bd", DT, USE_FP32R, C_BLK)
    if key in _cache:
        return _cache[key]
    f32 = mybir.dt.float32
    f32r = mybir.dt.float32r
    bf16 = mybir.dt.bfloat16
    if DT == "bf16":
        iodt = bf16  # dram + sbuf dtype
        mmdt = bf16
    else:
        iodt = f32
        mmdt = f32r if USE_FP32R else f32
    nc = bacc.Bacc(
        "TRN2", target_bir_lowering=False, debug=False, num_devices=N_CORES
    )
    xt_d = nc.dram_tensor("xt", [DIM, SHARD], iodt, kind="ExternalInput")
    s_d = [
        nc.dram_tensor(f"s{r}", [P, P], iodt, kind="ExternalInput") for r in range(4)
    ]
    out_d = nc.dram_tensor("outt", [DIM, SHARD], iodt, kind="ExternalOutput")

    with ExitStack() as ctx:
        tc = ctx.enter_context(tile.TileContext(nc))
        wpool = ctx.enter_context(tc.tile_pool(name="w", bufs=1))
        xpool = ctx.enter_context(tc.tile_pool(name="xin", bufs=2))
        opool = ctx.enter_context(tc.tile_pool(name="oout", bufs=2))
        # 2-bank psum tiles: two matmuls fill halves, one wide copy drains
        pspool = ctx.enter_context(tc.tile_pool(name="ps", bufs=4, space="PSUM"))

        ssb = []
        for r in range(4):
            t = wpool.tile([P, P], mmdt, tag=f"s{r}")
            if mmdt == f32r:
                stg = wpool.tile([P, P], f32, tag=f"sstg{r}")
                nc.scalar.dma_start(stg[:], s_d[r][:])
                nc.vector.tensor_copy(t[:], stg[:])
            else:
                nc.scalar.dma_start(t[:], s_d[r][:])
            ssb.append(t)

        ci = 0  # copy round-robin Vector/Scalar
        for bi, (c0, cw) in enumerate(_blocks()):
            xins = []
            for k, (ro, fw, si) in enumerate(CHUNKS_BD):
                xin = xpool.tile([P, C_BLK], mmdt, tag=f"x{k}")
                src = xt_d[ro : ro + fw, c0 : c0 + cw]
                # input stays on Sync (keeps prefetch decoupled from
                # output-side dependencies)
                eng = nc.sync
                eng.dma_start(
                    xin[:fw, :cw], src.bitcast(mmdt) if mmdt == f32r else src
                )
                xins.append(xin)
            for k, (ro, fw, si) in enumerate(CHUNKS_BD):
                xout = opool.tile([P, C_BLK], iodt, tag=f"o{k}")
                for t0 in range(0, cw, 1024):
                    tw = min(1024, cw - t0)
                    ps = pspool.tile([P, 1024], f32, tag="ps")
                    for g0 in range(0, tw, 512):
                        gw = min(512, tw - g0)
                        nc.tensor.matmul(
                            ps[:fw, g0 : g0 + gw],
                            ssb[si][:fw, :fw],
                            xins[k][:fw, t0 + g0 : t0 + g0 + gw],
                            start=True,
                            stop=True,
                        )
                    if ci % 2 == 0:
                        nc.vector.tensor_copy(
                            xout[:fw, t0 : t0 + tw], ps[:fw, :tw]
                        )
                    else:
                        nc.scalar.copy(xout[:fw, t0 : t0 + tw], ps[:fw, :tw])
                    ci += 1
                oeng = nc.scalar if k % 2 == 0 else nc.gpsimd
                oeng.dma_start(
                    out_d[ro : ro + fw, c0 : c0 + cw], xout[:fw, :cw]
                )

    nc.compile()
    _cache[key] = nc
    return nc


def _deinterleave(xs):
    """[S, 1056] -> de-interleaved, transposed [1056, S] (keeps dtype)."""
    out = np.empty((DIM, xs.shape[0]), xs.dtype)
    for off, m, d in SEGS:
        out[off : off + m * d] = (
            xs[:, off : off + m * d].reshape(-1, m, d).transpose(2, 1, 0).reshape(m * d, -1)
        )
    return out


def _reinterleave(ot):
    """[1056, S] de-interleaved -> natural f32 [S, 1056]."""
    S = ot.shape[1]
    out = np.empty((S, DIM), np.float32)
    for off, m, d in SEGS:
        out[:, off : off + m * d] = (
            ot[off : off + m * d]
            .reshape(d, m, S)
            .transpose(2, 1, 0)
            .reshape(S, m * d)
            .astype(np.float32)
        )
    return out


def _stationaries(ws, dtype):
    reps = [1, 1, 2, 4]
    out = []
    for w, k in zip(ws, reps):
        w = np.asarray(w, np.float32)
        m = w.shape[0]
        bd = np.zeros((P, P), np.float32)
        for a in range(k):
            bd[a * m : (a + 1) * m, a * m : (a + 1) * m] = w
        out.append(np.ascontiguousarray(bd.astype(dtype)))
    return out


last_result = None  # BassKernelResults of the most recent run (for profiling)

MODE = os.environ.get("KERNEL_MODE", "bd")


def kernel(x, w0, w1, w2, w3):
    global last_result
    x = np.asarray(x, dtype=np.float32)
    trace = os.environ.get("KERNEL_TRACE", "0") == "1"
    assert MODE == "bd", MODE
    if DT == "bf16":
        from ml_dtypes import bfloat16

        hostdt = bfloat16
    else:
        hostdt = np.float32
    nc = _build_bd()
    ss = _stationaries([w0, w1, w2, w3], hostdt)
    if hostdt is not np.float32:
        x = x.astype(hostdt)
    in_maps = []
    for c in range(N_CORES):
        m = {"xt": _deinterleave(x[c * SHARD : (c + 1) * SHARD])}
        for r in range(4):
            m[f"s{r}"] = ss[r]
        in_maps.append(m)
    last_result = run_bass_kernel_spmd(
        nc, in_maps, core_ids=list(range(N_CORES)), trace=trace
    )
    return np.ascontiguousarray(
        np.concatenate(
            [_reinterleave(r["outt"]) for r in last_result.results], axis=0
        )
    )


# revision 20
# speedup vs baseline: 1.1051x; 1.0099x over previous
"""Equivariant block-diagonal linear (128x0e+128x1o+64x2e+32x3o) on 8 trn2 cores.

Strategy (mode "bd", default):
  - Data-parallel: x [50000, 1056] row-sharded into 8x [6250, 1056].
  - Host de-interleaves each irrep: feature (off + u*d + i) -> row (off + i*mul + u)
    and transposes, so the device sees xt [1056, SHARD] where every irrep
    component i occupies a contiguous [mul, SHARD] row band.
  - Each 128-row chunk of xt then needs exactly ONE matmul with a small
    block-diagonal stationary blockdiag(w_r, ...) [fw, fw] - no accumulation
    chains, no kron(w, I_d) padding.
  - Per col-block of 1024 nodes: 9 input DMAs (4KB/partition lines), 9 matmuls
    per 512-col group, PSUM->SBUF copies on Vector, 9 output DMAs on Scalar.
    Input DMA issue on Sync/GpSimd so prefetch never queues behind outputs.
  - Host re-interleaves the gathered [1056, SHARD] outputs.
"""

import os
from contextlib import ExitStack

import numpy as np

import concourse.bass as bass
import concourse.tile as tile
from concourse import bacc, mybir
from concourse.bass_utils import run_bass_kernel_spmd

N_NODES = 50000
DIM = 1056
N_CORES = 8
SHARD = N_NODES // N_CORES  # 6250
P = 128

IRREPS = [(128, 0), (128, 1), (64, 2), (32, 3)]
# (offset, mul, d) per irrep in the feature axis
SEGS = [(0, 128, 1), (128, 128, 3), (512, 64, 5), (832, 32, 7)]

USE_FP32R = os.environ.get("KERNEL_FP32R", "1") == "1"
# device I/O + matmul dtype: bf16 halves DMA bytes (kernel is HBM-bound);
# rel err ~3e-3 vs the 2e-2 gate. "fp32r" keeps full-precision I/O.
DT = os.environ.get("KERNEL_DT", "bf16")

_cache = {}

# ---------------------------------------------------------------------------
# Mode "bd": de-interleaved rows; one matmul per 128-row chunk with a
# block-diagonal stationary.
# (row_off, rows, stationary_idx) - stationary s{si}[:rows, :rows]
CHUNKS_BD = [
    (0, 128, 0),
    (128, 128, 1),
    (256, 128, 1),
    (384, 128, 1),
    (512, 128, 2),
    (640, 128, 2),
    (768, 64, 2),
    (832, 128, 3),
    (960, 96, 3),
]
C_BLK = int(os.environ.get("KERNEL_CBLK", "2048" if DT == "bf16" else "1024"))


def _blocks():
    # small last block -> short post-input drain tail.
    sizes = None
    if sizes is None or sum(sizes) != SHARD:
        sizes = []
        left = SHARD
        while left > 0:
            sizes.append(min(C_BLK, left))
            left -= sizes[-1]
    out = []
    c0 = 0
    for cw in sizes:
        out.append((c0, cw))
        c0 += cw
    return out


def _build_bd():
    key = ("bd", DT, USE_FP32R, C_BLK)
    if key in _cache:
        return _cache[key]
    f32 = mybir.dt.float32
    f32r = mybir.dt.float32r
    bf16 = mybir.dt.bfloat16
    if DT == "bf16":
        iodt = bf16  # dram + sbuf dtype
        mmdt = bf16
    else:
        iodt = f32
        mmdt = f32r if USE_FP32R else f32
    nc = bacc.Bacc(
        "TRN2", target_bir_lowering=False, debug=False, num_devices=N_CORES
    )
    xt_d = nc.dram_tensor("xt", [DIM, SHARD], iodt, kind="ExternalInput")
    s_d = [
        nc.dram_tensor(f"s{r}", [P, P], iodt, kind="ExternalInput") for r in range(4)
    ]
    out_d = nc.dram_tensor("outt", [DIM, SHARD], iodt, kind="ExternalOutput")

    with ExitStack() as ctx:
        tc = ctx.enter_context(tile.TileContext(nc))
        wpool = ctx.enter_context(tc.tile_pool(name="w", bufs=1))
        xpool = ctx.enter_context(
            tc.tile_pool(name="xin", bufs=int(os.environ.get("KERNEL_XBUFS", "3")))
        )
        opool = ctx.enter_context(tc.tile_pool(name="oout", bufs=2))
        # 2-bank psum tiles: two matmuls fill halves, one wide copy drains
        pspool = ctx.enter_context(tc.tile_pool(name="ps", bufs=4, space="PSUM"))

        ssb = []
        for r in range(4):
            t = wpool.tile([P, P], mmdt, tag=f"s{r}")
            if mmdt == f32r:
                stg = wpool.tile([P, P], f32, tag=f"sstg{r}")
                nc.scalar.dma_start(stg[:], s_d[r][:])
                nc.vector.tensor_copy(t[:], stg[:])
            else:
                nc.scalar.dma_start(t[:], s_d[r][:])
            ssb.append(t)

        ci = 0  # copy round-robin Vector/Scalar
        for bi, (c0, cw) in enumerate(_blocks()):
            xins = []
            for k, (ro, fw, si) in enumerate(CHUNKS_BD):
                xin = xpool.tile([P, C_BLK], mmdt, tag=f"x{k}")
                src = xt_d[ro : ro + fw, c0 : c0 + cw]
                # input stays on Sync (keeps prefetch decoupled from
                # output-side dependencies)
                eng = nc.sync
                eng.dma_start(
                    xin[:fw, :cw], src.bitcast(mmdt) if mmdt == f32r else src
                )
                xins.append(xin)
            for k, (ro, fw, si) in enumerate(CHUNKS_BD):
                xout = opool.tile([P, C_BLK], iodt, tag=f"o{k}")
                for t0 in range(0, cw, 1024):
                    tw = min(1024, cw - t0)
                    ps = pspool.tile([P, 1024], f32, tag="ps")
                    for g0 in range(0, tw, 512):
                        gw = min(512, tw - g0)
                        nc.tensor.matmul(
                            ps[:fw, g0 : g0 + gw],
                            ssb[si][:fw, :fw],
                            xins[k][:fw, t0 + g0 : t0 + g0 + gw],
                            start=True,
                            stop=True,
                        )
                    if ci % 2 == 0:
                        nc.vector.tensor_copy(
                            xout[:fw, t0 : t0 + tw], ps[:fw, :tw]
                        )
                    else:
                        nc.scalar.copy(xout[:fw, t0 : t0 + tw], ps[:fw, :tw])
                    ci += 1
                oeng = nc.scalar if k % 2 == 0 else nc.gpsimd
                oeng.dma_start(
                    out_d[ro : ro + fw, c0 : c0 + cw], xout[:fw, :cw]
                )

    nc.compile()
    _cache[key] = nc
    return nc


def _deinterleave(xs):
    """[S, 1056] -> de-interleaved, transposed [1056, S] (keeps dtype)."""
    out = np.empty((DIM, xs.shape[0]), xs.dtype)
    for off, m, d in SEGS:
        out[off : off + m * d] = (
            xs[:, off : off + m * d].reshape(-1, m, d).transpose(2, 1, 0).reshape(m * d, -1)
        )
    return out


def _reinterleave(ot):
    """[1056, S] de-interleaved -> natural f32 [S, 1056]."""
    S = ot.shape[1]
    out = np.empty((S, DIM), np.float32)
    for off, m, d in SEGS:
        out[:, off : off + m * d] = (
            ot[off : off + m * d]
            .reshape(d, m, S)
            .transpose(2, 1, 0)
            .reshape(S, m * d)
            .astype(np.float32)
        )
    return out


def _stationaries(ws, dtype):
    reps = [1, 1, 2, 4]
    out = []
    for w, k in zip(ws, reps):
        w = np.asarray(w, np.float32)
        m = w.shape[0]
        bd = np.zeros((P, P), np.float32)
        for a in range(k):
            bd[a * m : (a + 1) * m, a * m : (a + 1) * m] = w
        out.append(np.ascontiguousarray(bd.astype(dtype)))
    return out


last_result = None  # BassKernelResults of the most recent run (for profiling)

MODE = os.environ.get("KERNEL_MODE", "bd")


def kernel(x, w0, w1, w2, w3):
    global last_result
    x = np.asarray(x, dtype=np.float32)
    trace = os.environ.get("KERNEL_TRACE", "0") == "1"
    assert MODE == "bd", MODE
    if DT == "bf16":
        from ml_dtypes import bfloat16

        hostdt = bfloat16
    else:
        hostdt = np.float32
    nc = _build_bd()
    ss = _stationaries([w0, w1, w2, w3], hostdt)
    if hostdt is not np.float32:
        x = x.astype(hostdt)
    in_maps = []
    for c in range(N_CORES):
        m = {"xt": _deinterleave(x[c * SHARD : (c + 1) * SHARD])}
        for r in range(4):
            m[f"s{r}"] = ss[r]
        in_maps.append(m)
    last_result = run_bass_kernel_spmd(
        nc, in_maps, core_ids=list(range(N_CORES)), trace=trace
    )
    return np.ascontiguousarray(
        np.concatenate(
            [_reinterleave(r["outt"]) for r in last_result.results], axis=0
        )
    )
